# revision 60
# baseline (speedup 1.0000x reference)
"""Trainium2 Bass kernel for nn_CrossAttentionHead.

Reference computation (B=64, C=512, H=W=28, N=784):
    att   = sigmoid(conv7x7([mean_c(x); max_c(x)]))          # [B,1,H,W]
    q     = x * att;  k = Wk x + bk;  v = Wv x + bv          # [B,C,N]
    E     = q^T k;  A = softmax(E, axis=-1)                  # [B,N,N]
    out   = mean_{h,w}(gamma * (V A^T) + x)                  # [B,C]

Exact algebraic restructuring:
  * trailing spatial mean is linear -> out[c] = gamma*(Wv (X s) + bv) + xmean
    with s[m] = (1/N) sum_n A[n,m]  (sum_m s[m] == 1 folds bv through).
  * k's bias adds a per-row constant to E -> drops out of softmax exactly.
  * att>0 folded into the softmax exp as per-row scale/bias on the ACT engine.

Layout/engine strategy:
  * x ships three ways: bf16 row-major [C, N] chunks (Wk / energy matmuls),
    bf16 position-major [112, NT, C] (xT: channel-max becomes a pairwise-max
    tree + free-axis reduce instead of the gpsimd partition reduce that was
    84% of the old kernel; X@s becomes tiny PE matmuls), and fp32 row-major
    used only for an exact fp32 xmean accumulation (the xmean term dominates
    the output, so it gets full precision).
  * channel-sum for the spatial attention is a ones-vector matmul on PE.
  * s is accumulated in position-major [112, NT] directly by per-tile PE
    matmuls (stationary = exp tile, moving = per-row 1/Z), killing the
    DRAM broadcast bounce + the big vector STT accumulations.
  * softmax uses a fixed log-domain offset (-55*att) instead of a per-row
    max: the measured exp-argument margins are >20 in log space both ways,
    and it removes the PSUM->DVE reduce from the E->exp critical path.
  * sigmoid is computed as 1/(1+exp(-z)) so the ACT engine never swaps its
    activation table (a Sigmoid op costs two 1.3us ACT_TABLE_LOADs/batch).
  * all large matmuls run in bf16 (fp32 "HIGH" mode is ~3.7x slower); the
    conv/plane front-end is bf16 too (fp32 1-col matmuls double-pass).
  * K has its own PSUM pool and runs at the END of iteration b-1, covering
    the exp(nt5)/exp(nt6) latency; xs(b-1) covers the s_mv copy; a warm-up
    burst opens the HAM clock gate while the first x DMA is in flight.

Sharding: pure data parallel over batch, 8 batches per NeuronCore x 8 cores.
"""

import numpy as np

import bass_rust
import concourse.bass as bass
import concourse.tile as tile
from concourse import mybir
from concourse.bass_utils import run_bass_kernel_spmd

AL = mybir.AluOpType
AF = mybir.ActivationFunctionType
AX = mybir.AxisListType
F32 = mybir.dt.float32
BF16 = mybir.dt.bfloat16

B, C, H, W = 64, 512, 28, 28
N = H * W            # 784
NCORES = 8
BPC = B // NCORES    # batches per core
CCH = C // 128       # 4 channel chunks of 128
NTILE = 112          # position-tile = 4 rows of 28; 7 tiles cover N
NT = N // NTILE      # 7
PAD = 3
WP = W + 2 * PAD     # 34
NPADF = WP * WP      # 1156 padded positions
KS = 7
TAPS = 2 * KS * KS   # 98
MAXSHIFT = (KS - 1) * WP + (KS - 1)  # 210
FPADW = NPADF + MAXSHIFT             # padded plane row width (zero margin)
NH0, NH1 = 512, N - 512              # energy column split per PSUM bank


class _TC(tile.TileContext):
    """TileContext whose end-of-kernel drain spreads its semaphore waits
    across nop instructions: this walrus build rejects >2 sync waits on a
    single CTRL instruction."""

    def _drain_and_barrier(self, tick_clock, wait_clock):
        nc = self.nc
        probe = nc.sync.nop()
        wait_clock.add_sem_waits(
            probe.ins, bass_rust.ScopedClock({None: tick_clock.global_clock})
        )
        si = probe.ins.sync_info
        waits = list(si.on_wait or [])
        si.on_wait = waits[:1]
        probe.ins.sync_info = si
        for w in waits[1:]:
            n2 = nc.sync.nop(nofuse=True)
            si2 = n2.ins.sync_info
            if si2 is None:
                si2 = mybir.SyncInfo(on_wait=[w], on_update=[])
            else:
                si2.on_wait = [w]
            n2.ins.sync_info = si2
        nc.sync.drain()
        nc.all_engine_barrier()
        assert self.sems is not None
        popped = nc._tile_sem_poison_stack.pop()
        assert popped is self._sem_poison
        nc.clear_and_free_semaphores(list(self.sems.allocated().values()))
        nc.all_engine_barrier()


def _spill_waits(nc, cap=1):
    """This walrus build rejects instructions carrying more than ~1 sync
    wait.  Move excess waits onto NoOp instructions inserted just before the
    owning instruction on the same engine."""
    ctr = 0
    for f in nc.m.functions:
        for bb in f.blocks:
            out = []
            for inst in bb.instructions:
                si = inst.sync_info
                waits = list(si.on_wait) if si and si.on_wait else []
                if len(waits) > cap:
                    for w in waits[cap:]:
                        ctr += 1
                        nop = mybir.InstNoOp(name=f"wspill-{ctr}", ins=[], outs=[])
                        nop.engine = inst.engine
                        nop.sync_info = mybir.SyncInfo(on_wait=[w], on_update=[])
                        out.append(nop)
                    si.on_wait = waits[:cap]
                    inst.sync_info = si
                out.append(inst)
            bb.instructions = out


def _build():
    nc = bass.Bass()
    xd = nc.dram_tensor("x", (BPC, C, N), BF16, kind="ExternalInput")
    xfd = nc.dram_tensor("xf", (BPC, C, N), F32, kind="ExternalInput")
    xtd = nc.dram_tensor("xt", (BPC, NT, NTILE, C), BF16, kind="ExternalInput")
    wkd = nc.dram_tensor("wkT", (C, C), BF16, kind="ExternalInput")  # [cin, cout]
    wvd = nc.dram_tensor("wvT", (C, C), BF16, kind="ExternalInput")  # [cin, cout]
    sad = nc.dram_tensor("sa98", (TAPS, 16), BF16, kind="ExternalInput")
    gbd = nc.dram_tensor("gbvg", (128, CCH + 1), F32, kind="ExternalInput")
    idd = nc.dram_tensor("ident", (128, 128), BF16, kind="ExternalInput")
    outd = nc.dram_tensor("out", (C, BPC), F32, kind="ExternalOutput")

    with _TC(nc) as tc:
        _emit_body(nc, tc, xd, xfd, xtd, wkd, wvd, sad, gbd, idd, outd)
    _spill_waits(nc)
    return nc


def _emit_body(nc, tc, xd, xfd, xtd, wkd, wvd, sad, gbd, idd, outd):
    import contextlib

    ctx = contextlib.ExitStack()
    with ctx:
        consts = ctx.enter_context(tc.tile_pool(name="consts", bufs=1))
        xpool = ctx.enter_context(tc.tile_pool(name="xpool", bufs=3))
        xfp = ctx.enter_context(tc.tile_pool(name="xfp", bufs=2))
        xtp = ctx.enter_context(tc.tile_pool(name="xtp", bufs=3))
        kpool = ctx.enter_context(tc.tile_pool(name="kpool", bufs=2))
        epool = ctx.enter_context(tc.tile_pool(name="epool", bufs=2))
        cpool = ctx.enter_context(tc.tile_pool(name="cpool", bufs=2))
        small = ctx.enter_context(tc.tile_pool(name="small", bufs=2))
        scratch = ctx.enter_context(tc.tile_pool(name="scratch", bufs=1))
        ps_big = ctx.enter_context(tc.tile_pool(name="ps_big", bufs=2, space="PSUM"))
        kps = ctx.enter_context(tc.tile_pool(name="kps", bufs=2, space="PSUM"))
        ps_misc = ctx.enter_context(tc.tile_pool(name="ps_misc", bufs=1, space="PSUM"))
        dram_p = ctx.enter_context(tc.tile_pool(name="dram_p", bufs=1, space="DRAM"))

        # ---- constants (DMAs for the big ones are issued late, in the
        # prologue, so the x loads win the serial sync-dispatch queue) ----
        ones_bf = consts.tile([128, 1], BF16, tag="ones_bf")
        nc.vector.memset(ones_bf, 1.0)
        warm = consts.tile([128, 8], BF16, tag="warm")
        nc.vector.memset(warm, 0.0)

        wk_sb = consts.tile([128, CCH, C], BF16, tag="wk")
        wv_sb = consts.tile([128, CCH, C], BF16, tag="wv")
        sa_sb = consts.tile([TAPS, 16], BF16, tag="sa")
        gb_sb = consts.tile([128, CCH + 1], F32, tag="gb")
        id_sb = consts.tile([128, 128], BF16, tag="ident")

        # moving operand of the X@s matmuls (written per batch from PSUM)
        s_mv = consts.tile([NTILE, NT], BF16, tag="s_mv")

        # zero-bordered feature planes live in DRAM (borders written once)
        zsb = consts.tile([2, FPADW], BF16, tag="zsb")
        nc.vector.memset(zsb, 0.0)
        fds = [
            dram_p.tile([2, FPADW], BF16, tag=f"fd{i}", name=f"fd{i}")
            for i in range(2)
        ]

        # accumulators across batches: [128, chunk*BPC]
        xs_acc = consts.tile([128, CCH * BPC], BF16, tag="xs_acc")
        xm_acc = consts.tile([128, CCH * BPC], F32, tag="xm_acc")
        xm_dump = scratch.tile([128, N], BF16, tag="xm_dump")

        # ---- PE warm-up: opens the HAM clock gate while the first x DMA
        # is still in flight (depends only on the memsets above) ----
        p_warm = ps_misc.tile([128, 8], F32, tag="psx")
        for _ in range(224):
            nc.tensor.matmul(
                p_warm[0:1, 0:8],
                ones_bf,
                warm,
                start=True,
                stop=True,
                skip_group_check=True,
            )

        xb_t = {}
        xf_t = {}
        xt_t = {}
        mrow_t = {}
        col_t = {}
        att_t = {}

        def load_x(b):
            xb = xpool.tile([128, CCH, N], BF16, tag="xb")
            xv = xd[b].rearrange("(ci c) n -> c ci n", c=128)
            for h in range(2):
                nc.sync.dma_start(xb[:, 2 * h : 2 * h + 2, :], xv[:, 2 * h : 2 * h + 2, :])
            xb_t[b] = xb
            xt = xtp.tile([NTILE, NT, C], BF16, tag="xt")
            xv2 = xtd[b].rearrange("nt p c -> p nt c")
            for h in range(2):
                sl = slice(4 * h, 4 * h + 4) if h == 0 else slice(4, NT)
                nc.sync.dma_start(xt[:, sl, :], xv2[:, sl, :])
            xt_t[b] = xt

        def load_xf(b):
            xf = xfp.tile([128, CCH, N], F32, tag="xf")
            xv = xfd[b].rearrange("(ci c) n -> c ci n", c=128)
            for h in range(2):
                nc.sync.dma_start(xf[:, 2 * h : 2 * h + 2, :], xv[:, 2 * h : 2 * h + 2, :])
            xf_t[b] = xf

        def front_sum(b):
            """channel-sum via ones-matmul -> srow [1,N] -> sum plane (PE+DVE)."""
            xb = xb_t[b]
            p_rowA = kps.tile([1, NH0], F32, tag="pk", name="p_rowA")
            p_rowB = kps.tile([1, NH1], F32, tag="pk", name="p_rowB")
            for pdst, lo, hi in ((p_rowA, 0, NH0), (p_rowB, NH0, N)):
                for ci in range(CCH):
                    nc.tensor.matmul(
                        pdst[0:1, 0 : hi - lo],
                        ones_bf,
                        xb[:, ci, lo:hi],
                        start=(ci == 0),
                        stop=(ci == CCH - 1),
                        skip_group_check=True,
                    )
            srow = small.tile([1, N], BF16, tag="srow")
            nc.vector.tensor_copy(srow[0:1, 0:NH0], p_rowA[0:1, 0:NH0])
            nc.vector.tensor_copy(srow[0:1, NH0:N], p_rowB[0:1, 0:NH1])
            fd = fds[b % 2]
            dst = bass.AP(
                tensor=fd.tensor,
                offset=fd.offset + 0 * FPADW + PAD * WP + PAD,
                ap=[[WP, H], [1, W]],
            )
            nc.sync.dma_start(dst, srow[0:1, :].rearrange("p (h w) -> p h w", w=W))

        mx_t = {}

        def front_max_tile(b, j):
            """channel-max over xT: 3 pairwise bf16 maxes (2x DVE mode) + one
            fused free-axis reduce; spread over the first 4 nt slots."""
            xt = xt_t[b]
            if j == 0:
                mx_t[b] = [
                    small.tile([NTILE, NT, 128], BF16, tag=f"mx{i}", name="mx")
                    for i in range(3)
                ]
                mrow_t[b] = small.tile([NTILE, NT], BF16, tag="mrow", name="mrow")
            if j > 3:
                return
            mxa, mxb, mxc = mx_t[b]
            if j == 0:
                nc.vector.tensor_max(mxa, xt[:, :, 0:128], xt[:, :, 128:256])
            elif j == 1:
                nc.vector.tensor_max(mxb, xt[:, :, 256:384], xt[:, :, 384:512])
            elif j == 2:
                nc.vector.tensor_max(mxc, mxa, mxb)
            elif j == 3:
                nc.vector.reduce_max(mrow_t[b][:, 0:NT], mxc, axis=AX.X)
                mx_t.pop(b)

        def front_maxplane(b):
            """[112, NT] -> [NT, 112] via identity matmul, then one clean
            28-descriptor DMA into the padded max plane."""
            mrow = mrow_t.pop(b)
            p_mt = ps_misc.tile([8, 112], F32, tag="psx")
            nc.tensor.matmul(
                p_mt[0:NT, 0:NTILE],
                mrow[:, 0:NT],
                id_sb[0:NTILE, 0:NTILE],
                start=True,
                stop=True,
                skip_group_check=True,
            )
            mrowT = small.tile([8, NTILE], BF16, tag="mrowT")
            nc.scalar.copy(mrowT[0:NT, :], p_mt[0:NT, 0:NTILE])
            fd = fds[b % 2]
            dst = bass.AP(
                tensor=fd.tensor,
                offset=fd.offset + 1 * FPADW + PAD * WP + PAD,
                ap=[[4 * WP, NT], [WP, 4], [1, W]],
            )
            nc.sync.dma_start(dst, mrowT[0:NT, :])

        colg_t = {}

        def front_col_half(b, c2):
            """im2col gather of one channel plane -> col[(c2,kh,kw), pad-n].
            The sum half (c2=0) is issued mid-loop, right after the sum plane
            lands, so only the max half remains on the iteration-end chain."""
            fd = fds[b % 2]
            if c2 == 0:
                colg_t[b] = cpool.tile([TAPS, NPADF], BF16, tag="col", name="col")
            col = colg_t[b]
            src = bass.AP(
                tensor=fd.tensor,
                offset=fd.offset + c2 * FPADW,
                ap=[[WP, KS], [1, KS], [1, NPADF]],
            )
            dst = bass.AP(
                tensor=col.tensor,
                offset=col.offset + c2 * (KS * KS) * NPADF,
                ap=[[NPADF, KS * KS], [1, 1], [1, NPADF]],
            )
            nc.sync.dma_start(dst, src)

        def front_col(b):
            """one SBUF->SBUF DMA selects the 784 valid positions."""
            col = colg_t.pop(b)
            col2 = cpool.tile([TAPS, N], BF16, tag="col2")
            src = bass.AP(
                tensor=col.tensor,
                offset=col.offset,
                ap=[[NPADF, TAPS], [WP, H], [1, W]],
            )
            nc.sync.dma_start(col2[:].rearrange("p (h w) -> p h w", w=W), src)
            col_t[b] = col2

        def front_conv(b):
            """7x7 conv as 7 one-column matmuls + sigmoid via exp (the ACT
            engine stays on the Exp table: a Sigmoid would force two 1.3us
            ACT_TABLE_LOADs per batch)."""
            col2 = col_t.pop(b)
            p_att = ps_misc.tile([128, 8], F32, tag="psx")
            att = small.tile([NTILE, NT], F32, tag="att")
            for nt in range(NT):
                nc.tensor.matmul(
                    p_att[:NTILE, nt : nt + 1],
                    col2[:, nt * NTILE : (nt + 1) * NTILE],
                    sa_sb[:, 0:1],
                    start=True,
                    stop=True,
                    skip_group_check=True,
                )
            nc.scalar.activation(
                att[:, 0:NT], p_att[:NTILE, 0:NT], AF.Exp, scale=-1.0
            )
            nc.vector.tensor_scalar_add(att[:, 0:NT], att[:, 0:NT], 1.0)
            nc.vector.reciprocal(att[:, 0:NT], att[:, 0:NT])
            # fixed log-domain offset: E stays within exp's fp32 range for
            # any plausible N(0,1)-scaled input (measured margins > 20 both
            # sides), so no per-row max reduction is needed at all
            bias = small.tile([NTILE, NT], F32, tag="bias_t")
            nc.vector.tensor_scalar_mul(bias[:, 0:NT], att[:, 0:NT], -55.0)
            att_t[b] = (att, bias)

        def finish(b):
            """X @ s for batch b: s PSUM->SBUF, 28 tiny matmuls, acc copy."""
            nc.vector.tensor_copy(s_mv[:, 0:NT], pst_t.pop(b)[:, 0:NT])
            xt = xt_t.pop(b)
            p_xs = ps_misc.tile([128, 8], F32, tag="psx")
            for co in range(CCH):
                for mt in range(NT):
                    nc.tensor.matmul(
                        p_xs[:, co : co + 1],
                        xt[:, mt, co * 128 : (co + 1) * 128],
                        s_mv[:, mt : mt + 1],
                        start=(mt == 0),
                        stop=(mt == NT - 1),
                        skip_group_check=True,
                    )
            xs_v = xs_acc[:].rearrange("p (co bb) -> p co bb", bb=BPC)
            nc.vector.tensor_copy(xs_v[:, :, b], p_xs[:, 0:CCH])
            return p_xs

        pst_t = {}
        k_t = {}

        def k_proj(b):
            """k = Wk x (PE, own PSUM pool) + PSUM->SBUF bf16 copies (ACT)."""
            xb = xb_t[b]
            k_sb = kpool.tile([128, CCH, N], BF16, tag="k_sb")
            for co in range(CCH):
                for lo, hi in ((0, NH0), (NH0, N)):
                    pk = kps.tile([128, NH0], F32, tag="pk")
                    for ci in range(CCH):
                        nc.tensor.matmul(
                            pk[:, 0 : hi - lo],
                            wk_sb[:, ci, co * 128 : (co + 1) * 128],
                            xb[:, ci, lo:hi],
                            start=(ci == 0),
                            stop=(ci == CCH - 1),
                        )
                    nc.scalar.copy(k_sb[:, co, lo:hi], pk[:, 0 : hi - lo])
            k_t[b] = k_sb

        # ---- prologue: x loads first, then consts in order of first use ----
        xv0 = xd[0].rearrange("(ci c) n -> c ci n", c=128)
        xb0 = xpool.tile([128, CCH, N], BF16, tag="xb", name="xb0")
        for h in range(2):
            nc.sync.dma_start(
                xb0[:, 2 * h : 2 * h + 2, :], xv0[:, 2 * h : 2 * h + 2, :]
            )
        xb_t[0] = xb0
        nc.sync.dma_start(wk_sb, wkd[:].rearrange("(ci c) o -> c ci o", c=128))
        xt0 = xtp.tile([NTILE, NT, C], BF16, tag="xt", name="xt0")
        xv2 = xtd[0].rearrange("nt p c -> p nt c")
        nc.sync.dma_start(xt0[:, 0:4, :], xv2[:, 0:4, :])
        nc.sync.dma_start(xt0[:, 4:NT, :], xv2[:, 4:NT, :])
        xt_t[0] = xt0
        for fd in fds:
            nc.sync.dma_start(fd, zsb)
        nc.sync.dma_start(id_sb, idd[:])
        nc.sync.dma_start(sa_sb, sad[:])
        load_x(1)
        load_xf(0)
        nc.sync.dma_start(gb_sb, gbd[:])
        nc.sync.dma_start(wv_sb, wvd[:].rearrange("(ci c) o -> c ci o", c=128))
        for j in range(4):
            front_max_tile(0, j)
        front_sum(0)
        k_proj(0)
        front_maxplane(0)
        front_col_half(0, 0)
        front_col_half(0, 1)
        front_col(0)

        for b in range(BPC):
            xb = xb_t.pop(b)
            if b + 2 < BPC:
                load_x(b + 2)
            if b + 1 < BPC:
                load_xf(b + 1)
            if b > 0:
                finish(b - 1)

            # conv for THIS batch: col(b) was gathered an iteration ago
            front_conv(b)
            k_sb = k_t.pop(b)

            att, bias_t = att_t.pop(b)
            xf = xf_t.pop(b)
            exp_sb = epool.tile([NTILE, NT, N], BF16, tag="exp_sb")
            r_bf = small.tile([NTILE, NT], BF16, tag="r_bf")
            zsum = small.tile([NTILE, NT], F32, tag="zsum")
            p_stile = ps_misc.tile([NTILE, 8], F32, tag="p_stile")
            pst_t[b] = p_stile

            def s_mms(nt):
                # s[m] accumulation, position-major: stationary = exp tile,
                # moving = per-row 1/Z. 7 one-column matmuls.
                for mt in range(NT):
                    nc.tensor.matmul(
                        p_stile[:, mt : mt + 1],
                        exp_sb[:, nt, mt * NTILE : (mt + 1) * NTILE],
                        r_bf[:, nt : nt + 1],
                        start=(nt == 0),
                        stop=(nt == NT - 1),
                        skip_group_check=True,
                    )

            # ---- energy + fused softmax ----
            for nt in range(NT):
                pe = ps_big.tile([128, 1024], F32, tag="pE")
                nsl = slice(nt * NTILE, (nt + 1) * NTILE)
                for lo, hi in ((0, NH0), (NH0, N)):
                    for ci in range(CCH):
                        nc.tensor.matmul(
                            pe[:NTILE, lo:hi],
                            xb[:, ci, nsl],
                            k_sb[:, ci, lo:hi],
                            start=(ci == 0),
                            stop=(ci == CCH - 1),
                        )
                if nt > 1:
                    s_mms(nt - 2)
                if nt == 3 and b + 1 < BPC:
                    front_sum(b + 1)
                    front_col_half(b + 1, 0)

                nc.scalar.activation(
                    exp_sb[:, nt, :],
                    pe[:NTILE, 0:N],
                    AF.Exp,
                    bias=bias_t[:, nt : nt + 1],
                    scale=att[:, nt : nt + 1],
                    accum_out=zsum[:, nt : nt + 1],
                )
                nc.vector.reciprocal(zsum[:, nt : nt + 1], zsum[:, nt : nt + 1])
                nc.vector.tensor_copy(r_bf[:, nt : nt + 1], zsum[:, nt : nt + 1])
                # next batch's channel-max rides the DVE slack, 1 tile per nt
                if b + 1 < BPC:
                    front_max_tile(b + 1, nt)

            # next batch's K projection fills the PE while exp(nt5)/exp(nt6)
            # and their reciprocals land on ACT/DVE
            if b + 1 < BPC:
                k_proj(b + 1)
            # exact fp32 xmean: only needed by the final tail, so it sits at
            # the very end of the iteration's DVE queue where a late xf DMA
            # cannot stall anything downstream
            for ci in range(CCH):
                acc = xm_acc[:, ci * BPC + b : ci * BPC + b + 1]
                if ci < 2:
                    nc.scalar.activation(
                        xm_dump, xf[:, ci, :], AF.Copy,
                        bias=0.0, scale=1.0 / N, accum_out=acc,
                    )
                else:
                    nc.vector.tensor_scalar(
                        out=xm_dump, in0=xf[:, ci, :],
                        scalar1=1.0 / N, scalar2=0.0,
                        op0=AL.mult, op1=AL.add, accum_out=acc,
                    )
            if b + 1 == BPC:
                # no next-batch K to fill the PE: bridge the exp(nt5)/exp(nt6)
                # latency with warm dummies so HAM stays open into the tail
                for _ in range(56):
                    nc.tensor.matmul(
                        p_warm[0:1, 0:8], ones_bf, warm,
                        start=True, stop=True, skip_group_check=True,
                    )
            s_mms(NT - 2)
            if b + 1 == BPC:
                for _ in range(40):
                    nc.tensor.matmul(
                        p_warm[0:1, 0:8], ones_bf, warm,
                        start=True, stop=True, skip_group_check=True,
                    )
            s_mms(NT - 1)
            if b + 1 < BPC:
                front_maxplane(b + 1)
                front_col_half(b + 1, 1)
                front_col(b + 1)

        finish(BPC - 1)

        # ---- tail: res = Wv^T @ XS ; out = res*(gamma/N) + (gamma*bv + xmean)
        res_all = scratch.tile([128, CCH, BPC], F32, tag="res_all")
        for co in range(CCH):
            pr = ps_big.tile([128, 1024], F32, tag="pE")
            for ci in range(CCH):
                nc.tensor.matmul(
                    pr[:, 0:BPC],
                    wv_sb[:, ci, co * 128 : (co + 1) * 128],
                    xs_acc[:, ci * BPC : (ci + 1) * BPC],
                    start=(ci == 0),
                    stop=(ci == CCH - 1),
                    skip_group_check=True,
                )
            t2 = scratch.tile([128, BPC], F32, tag=f"t2_{co}", name="t2")
            nc.vector.tensor_scalar_add(
                t2, xm_acc[:, co * BPC : (co + 1) * BPC], gb_sb[:, co : co + 1]
            )
            nc.vector.scalar_tensor_tensor(
                out=res_all[:, co, :],
                in0=pr[:, 0:BPC],
                scalar=gb_sb[:, CCH : CCH + 1],
                in1=t2,
                op0=AL.mult,
                op1=AL.add,
            )
        nc.sync.dma_start(
            outd[:].rearrange("(ci c) b -> c ci b", c=128), res_all
        )


_CACHE = {}


def _get_nc():
    if "nc" not in _CACHE:
        _CACHE["nc"] = _build()
    return _CACHE["nc"]


def kernel(x, sa_w, key_w, key_b, value_w, value_b, gamma, _trace=False):
    import ml_dtypes

    BF = ml_dtypes.bfloat16
    x = np.ascontiguousarray(np.asarray(x, dtype=np.float32)).reshape(B, C, N)
    sa_w = np.asarray(sa_w, dtype=np.float32)
    key_w = np.asarray(key_w, dtype=np.float32)
    value_w = np.asarray(value_w, dtype=np.float32)
    value_b = np.asarray(value_b, dtype=np.float32)
    gamma = float(np.asarray(gamma).reshape(-1)[0])

    # host-side parameter reshuffles (layout only / tiny folds)
    sa98 = sa_w.reshape(2, KS * KS).copy()
    sa98[0] *= 1.0 / C                      # channel-mean fold
    sa98 = np.repeat(sa98.reshape(TAPS, 1), 16, axis=1).astype(BF)
    sa98 = np.ascontiguousarray(sa98)
    wkT = np.ascontiguousarray(key_w.T.astype(BF))
    wvT = np.ascontiguousarray(value_w.T.astype(BF))
    gbvg = np.empty((128, CCH + 1), np.float32)
    gbvg[:, :CCH] = (gamma * value_b).reshape(CCH, 128).T
    gbvg[:, CCH] = gamma / N
    gbvg = np.ascontiguousarray(gbvg)
    ident = np.eye(128, dtype=np.float32).astype(BF)

    xbf = x.astype(BF)                                        # [B, C, N]
    xt = np.ascontiguousarray(
        xbf.reshape(B, C, NT, NTILE).transpose(0, 2, 3, 1)
    )                                                         # [B, NT, 112, C]

    nc = _get_nc()
    in_maps = []
    for i in range(NCORES):
        sl = slice(i * BPC, (i + 1) * BPC)
        in_maps.append(
            {
                "x": np.ascontiguousarray(xbf[sl]),
                "xf": np.ascontiguousarray(x[sl]),
                "xt": xt[sl],
                "wkT": wkT,
                "wvT": wvT,
                "sa98": sa98,
                "gbvg": gbvg,
                "ident": ident,
            }
        )
    r = run_bass_kernel_spmd(
        nc, in_maps, core_ids=list(range(NCORES)), trace=_trace
    )
    out = np.empty((B, C), np.float32)
    for i in range(NCORES):
        out[i * BPC : (i + 1) * BPC] = r.results[i]["out"].T
    if _trace:
        kernel.last_results = r
    return out


# revision 61
# speedup vs baseline: 1.0139x; 1.0139x over previous
"""Trainium2 Bass kernel for nn_CrossAttentionHead.

Reference computation (B=64, C=512, H=W=28, N=784):
    att   = sigmoid(conv7x7([mean_c(x); max_c(x)]))          # [B,1,H,W]
    q     = x * att;  k = Wk x + bk;  v = Wv x + bv          # [B,C,N]
    E     = q^T k;  A = softmax(E, axis=-1)                  # [B,N,N]
    out   = mean_{h,w}(gamma * (V A^T) + x)                  # [B,C]

Exact algebraic restructuring:
  * trailing spatial mean is linear -> out[c] = gamma*(Wv (X s) + bv) + xmean
    with s[m] = (1/N) sum_n A[n,m]  (sum_m s[m] == 1 folds bv through).
  * k's bias adds a per-row constant to E -> drops out of softmax exactly.
  * att>0 folded into the softmax exp as per-row scale/bias on the ACT engine.

Layout/engine strategy:
  * x ships three ways: bf16 row-major [C, N] chunks (Wk / energy matmuls),
    bf16 position-major [112, NT, C] (xT: channel-max becomes a pairwise-max
    tree + free-axis reduce instead of the gpsimd partition reduce that was
    84% of the old kernel; X@s becomes tiny PE matmuls), and fp32 row-major
    used only for an exact fp32 xmean accumulation (the xmean term dominates
    the output, so it gets full precision).
  * channel-sum for the spatial attention is a ones-vector matmul on PE.
  * s is accumulated in position-major [112, NT] directly by per-tile PE
    matmuls (stationary = exp tile, moving = per-row 1/Z), killing the
    DRAM broadcast bounce + the big vector STT accumulations.
  * softmax uses a fixed log-domain offset (-55*att) instead of a per-row
    max: the measured exp-argument margins are >20 in log space both ways,
    and it removes the PSUM->DVE reduce from the E->exp critical path.
  * sigmoid is computed as 1/(1+exp(-z)) so the ACT engine never swaps its
    activation table (a Sigmoid op costs two 1.3us ACT_TABLE_LOADs/batch).
  * all large matmuls run in bf16 (fp32 "HIGH" mode is ~3.7x slower); the
    conv/plane front-end is bf16 too (fp32 1-col matmuls double-pass).
  * K has its own PSUM pool and runs at the END of iteration b-1, covering
    the exp(nt5)/exp(nt6) latency; xs(b-1) covers the s_mv copy; a warm-up
    burst opens the HAM clock gate while the first x DMA is in flight.

Sharding: pure data parallel over batch, 8 batches per NeuronCore x 8 cores.
"""

import numpy as np

import bass_rust
import concourse.bass as bass
import concourse.tile as tile
from concourse import mybir
from concourse.bass_utils import run_bass_kernel_spmd

AL = mybir.AluOpType
AF = mybir.ActivationFunctionType
AX = mybir.AxisListType
F32 = mybir.dt.float32
BF16 = mybir.dt.bfloat16

B, C, H, W = 64, 512, 28, 28
N = H * W            # 784
NCORES = 8
BPC = B // NCORES    # batches per core
CCH = C // 128       # 4 channel chunks of 128
NTILE = 112          # position-tile = 4 rows of 28; 7 tiles cover N
NT = N // NTILE      # 7
PAD = 3
WP = W + 2 * PAD     # 34
NPADF = WP * WP      # 1156 padded positions
KS = 7
TAPS = 2 * KS * KS   # 98
MAXSHIFT = (KS - 1) * WP + (KS - 1)  # 210
FPADW = NPADF + MAXSHIFT             # padded plane row width (zero margin)
NH0, NH1 = 512, N - 512              # energy column split per PSUM bank


class _TC(tile.TileContext):
    """TileContext whose end-of-kernel drain spreads its semaphore waits
    across nop instructions: this walrus build rejects >2 sync waits on a
    single CTRL instruction."""

    def _drain_and_barrier(self, tick_clock, wait_clock):
        nc = self.nc
        probe = nc.sync.nop()
        wait_clock.add_sem_waits(
            probe.ins, bass_rust.ScopedClock({None: tick_clock.global_clock})
        )
        si = probe.ins.sync_info
        waits = list(si.on_wait or [])
        si.on_wait = waits[:1]
        probe.ins.sync_info = si
        for w in waits[1:]:
            n2 = nc.sync.nop(nofuse=True)
            si2 = n2.ins.sync_info
            if si2 is None:
                si2 = mybir.SyncInfo(on_wait=[w], on_update=[])
            else:
                si2.on_wait = [w]
            n2.ins.sync_info = si2
        nc.sync.drain()
        nc.all_engine_barrier()
        assert self.sems is not None
        popped = nc._tile_sem_poison_stack.pop()
        assert popped is self._sem_poison
        nc.clear_and_free_semaphores(list(self.sems.allocated().values()))
        nc.all_engine_barrier()


def _spill_waits(nc, cap=1):
    """This walrus build rejects instructions carrying more than ~1 sync
    wait.  Move excess waits onto NoOp instructions inserted just before the
    owning instruction on the same engine."""
    ctr = 0
    for f in nc.m.functions:
        for bb in f.blocks:
            out = []
            for inst in bb.instructions:
                si = inst.sync_info
                waits = list(si.on_wait) if si and si.on_wait else []
                if len(waits) > cap:
                    for w in waits[cap:]:
                        ctr += 1
                        nop = mybir.InstNoOp(name=f"wspill-{ctr}", ins=[], outs=[])
                        nop.engine = inst.engine
                        nop.sync_info = mybir.SyncInfo(on_wait=[w], on_update=[])
                        out.append(nop)
                    si.on_wait = waits[:cap]
                    inst.sync_info = si
                out.append(inst)
            bb.instructions = out


def _build():
    nc = bass.Bass()
    xd = nc.dram_tensor("x", (BPC, C, N), BF16, kind="ExternalInput")
    xfd = nc.dram_tensor("xf", (BPC, C, N), F32, kind="ExternalInput")
    xtd = nc.dram_tensor("xt", (BPC, NT, NTILE, C), BF16, kind="ExternalInput")
    wkd = nc.dram_tensor("wkT", (C, C), BF16, kind="ExternalInput")  # [cin, cout]
    wvd = nc.dram_tensor("wvT", (C, C), BF16, kind="ExternalInput")  # [cin, cout]
    sad = nc.dram_tensor("sa98", (TAPS, 16), BF16, kind="ExternalInput")
    gbd = nc.dram_tensor("gbvg", (128, CCH + 1), F32, kind="ExternalInput")
    idd = nc.dram_tensor("ident", (128, 128), BF16, kind="ExternalInput")
    outd = nc.dram_tensor("out", (C, BPC), F32, kind="ExternalOutput")

    with _TC(nc) as tc:
        _emit_body(nc, tc, xd, xfd, xtd, wkd, wvd, sad, gbd, idd, outd)
    _spill_waits(nc)
    return nc


def _emit_body(nc, tc, xd, xfd, xtd, wkd, wvd, sad, gbd, idd, outd):
    import contextlib

    ctx = contextlib.ExitStack()
    with ctx:
        consts = ctx.enter_context(tc.tile_pool(name="consts", bufs=1))
        xpool = ctx.enter_context(tc.tile_pool(name="xpool", bufs=3))
        xfp = ctx.enter_context(tc.tile_pool(name="xfp", bufs=2))
        xtp = ctx.enter_context(tc.tile_pool(name="xtp", bufs=3))
        kpool = ctx.enter_context(tc.tile_pool(name="kpool", bufs=2))
        epool = ctx.enter_context(tc.tile_pool(name="epool", bufs=2))
        cpool = ctx.enter_context(tc.tile_pool(name="cpool", bufs=2))
        small = ctx.enter_context(tc.tile_pool(name="small", bufs=2))
        scratch = ctx.enter_context(tc.tile_pool(name="scratch", bufs=1))
        ps_big = ctx.enter_context(tc.tile_pool(name="ps_big", bufs=2, space="PSUM"))
        kps = ctx.enter_context(tc.tile_pool(name="kps", bufs=2, space="PSUM"))
        ps_misc = ctx.enter_context(tc.tile_pool(name="ps_misc", bufs=1, space="PSUM"))
        dram_p = ctx.enter_context(tc.tile_pool(name="dram_p", bufs=1, space="DRAM"))

        # ---- constants (DMAs for the big ones are issued late, in the
        # prologue, so the x loads win the serial sync-dispatch queue) ----
        ones_bf = consts.tile([128, 1], BF16, tag="ones_bf")
        nc.vector.memset(ones_bf, 1.0)
        warm = consts.tile([128, 8], BF16, tag="warm")
        nc.vector.memset(warm, 0.0)

        wk_sb = consts.tile([128, CCH, C], BF16, tag="wk")
        wv_sb = consts.tile([128, CCH, C], BF16, tag="wv")
        sa_sb = consts.tile([TAPS, 16], BF16, tag="sa")
        gb_sb = consts.tile([128, CCH + 1], F32, tag="gb")
        id_sb = consts.tile([128, 128], BF16, tag="ident")

        # moving operand of the X@s matmuls (written per batch from PSUM)
        s_mv = consts.tile([NTILE, NT], BF16, tag="s_mv")

        # zero-bordered feature planes live in DRAM (borders written once)
        zsb = consts.tile([2, FPADW], BF16, tag="zsb")
        nc.vector.memset(zsb, 0.0)
        fds = [
            dram_p.tile([2, FPADW], BF16, tag=f"fd{i}", name=f"fd{i}")
            for i in range(2)
        ]

        # accumulators across batches: [128, chunk*BPC]
        xs_acc = consts.tile([128, CCH * BPC], BF16, tag="xs_acc")
        xm_acc = consts.tile([128, CCH * BPC], F32, tag="xm_acc")
        xm_dump = scratch.tile([128, N], BF16, tag="xm_dump")

        # ---- PE warm-up: opens the HAM clock gate while the first x DMA
        # is still in flight (depends only on the memsets above) ----
        p_warm = ps_misc.tile([128, 8], F32, tag="psx")
        for _ in range(224):
            nc.tensor.matmul(
                p_warm[0:1, 0:8],
                ones_bf,
                warm,
                start=True,
                stop=True,
                skip_group_check=True,
            )

        xb_t = {}
        xf_t = {}
        xt_t = {}
        mrow_t = {}
        col_t = {}
        att_t = {}

        def load_x(b):
            xb = xpool.tile([128, CCH, N], BF16, tag="xb")
            xv = xd[b].rearrange("(ci c) n -> c ci n", c=128)
            for h in range(2):
                nc.sync.dma_start(xb[:, 2 * h : 2 * h + 2, :], xv[:, 2 * h : 2 * h + 2, :])
            xb_t[b] = xb
            xt = xtp.tile([NTILE, NT, C], BF16, tag="xt")
            xv2 = xtd[b].rearrange("nt p c -> p nt c")
            for h in range(2):
                sl = slice(4 * h, 4 * h + 4) if h == 0 else slice(4, NT)
                nc.sync.dma_start(xt[:, sl, :], xv2[:, sl, :])
            xt_t[b] = xt

        def load_xf(b):
            xf = xfp.tile([128, CCH, N], F32, tag="xf")
            xv = xfd[b].rearrange("(ci c) n -> c ci n", c=128)
            for h in range(2):
                nc.sync.dma_start(xf[:, 2 * h : 2 * h + 2, :], xv[:, 2 * h : 2 * h + 2, :])
            xf_t[b] = xf

        def front_sum(b):
            """channel-sum via ones-matmul -> srow [1,N] -> sum plane (PE+DVE)."""
            xb = xb_t[b]
            p_rowA = kps.tile([1, NH0], F32, tag="pk", name="p_rowA")
            p_rowB = kps.tile([1, NH1], F32, tag="pk", name="p_rowB")
            for pdst, lo, hi in ((p_rowA, 0, NH0), (p_rowB, NH0, N)):
                for ci in range(CCH):
                    nc.tensor.matmul(
                        pdst[0:1, 0 : hi - lo],
                        ones_bf,
                        xb[:, ci, lo:hi],
                        start=(ci == 0),
                        stop=(ci == CCH - 1),
                        skip_group_check=True,
                    )
            srow = small.tile([1, N], BF16, tag="srow")
            nc.vector.tensor_copy(srow[0:1, 0:NH0], p_rowA[0:1, 0:NH0])
            nc.vector.tensor_copy(srow[0:1, NH0:N], p_rowB[0:1, 0:NH1])
            fd = fds[b % 2]
            dst = bass.AP(
                tensor=fd.tensor,
                offset=fd.offset + 0 * FPADW + PAD * WP + PAD,
                ap=[[WP, H], [1, W]],
            )
            nc.sync.dma_start(dst, srow[0:1, :].rearrange("p (h w) -> p h w", w=W))

        mx_t = {}

        def front_max_tile(b, j):
            """channel-max over xT: 3 pairwise bf16 maxes (2x DVE mode) + one
            fused free-axis reduce; spread over the first 4 nt slots."""
            xt = xt_t[b]
            if j == 0:
                mx_t[b] = [
                    small.tile([NTILE, NT, 128], BF16, tag=f"mx{i}", name="mx")
                    for i in range(3)
                ]
                mrow_t[b] = small.tile([NTILE, NT], BF16, tag="mrow", name="mrow")
            if j > 3:
                return
            mxa, mxb, mxc = mx_t[b]
            if j == 0:
                nc.vector.tensor_max(mxa, xt[:, :, 0:128], xt[:, :, 128:256])
            elif j == 1:
                nc.vector.tensor_max(mxb, xt[:, :, 256:384], xt[:, :, 384:512])
            elif j == 2:
                nc.vector.tensor_max(mxc, mxa, mxb)
            elif j == 3:
                nc.vector.reduce_max(mrow_t[b][:, 0:NT], mxc, axis=AX.X)
                mx_t.pop(b)

        def front_maxplane(b):
            """[112, NT] -> [NT, 112] via identity matmul, then one clean
            28-descriptor DMA into the padded max plane."""
            mrow = mrow_t.pop(b)
            p_mt = ps_misc.tile([8, 112], F32, tag="psx")
            nc.tensor.matmul(
                p_mt[0:NT, 0:NTILE],
                mrow[:, 0:NT],
                id_sb[0:NTILE, 0:NTILE],
                start=True,
                stop=True,
                skip_group_check=True,
            )
            mrowT = small.tile([8, NTILE], BF16, tag="mrowT")
            nc.scalar.copy(mrowT[0:NT, :], p_mt[0:NT, 0:NTILE])
            fd = fds[b % 2]
            dst = bass.AP(
                tensor=fd.tensor,
                offset=fd.offset + 1 * FPADW + PAD * WP + PAD,
                ap=[[4 * WP, NT], [WP, 4], [1, W]],
            )
            nc.sync.dma_start(dst, mrowT[0:NT, :])

        def front_col(b):
            """im2col gather: padded planes -> col[(c2,kh,kw), padded-n],
            then one SBUF->SBUF DMA selects the 784 valid positions."""
            fd = fds[b % 2]
            col = cpool.tile([TAPS, NPADF], BF16, tag="col")
            for c2 in range(2):
                src = bass.AP(
                    tensor=fd.tensor,
                    offset=fd.offset + c2 * FPADW,
                    ap=[[WP, KS], [1, KS], [1, NPADF]],
                )
                dst = bass.AP(
                    tensor=col.tensor,
                    offset=col.offset + c2 * (KS * KS) * NPADF,
                    ap=[[NPADF, KS * KS], [1, 1], [1, NPADF]],
                )
                nc.sync.dma_start(dst, src)
            col2 = cpool.tile([TAPS, N], BF16, tag="col2")
            src = bass.AP(
                tensor=col.tensor,
                offset=col.offset,
                ap=[[NPADF, TAPS], [WP, H], [1, W]],
            )
            nc.sync.dma_start(col2[:].rearrange("p (h w) -> p h w", w=W), src)
            col_t[b] = col2

        def front_conv(b):
            """7x7 conv as 7 one-column matmuls + sigmoid via exp (the ACT
            engine stays on the Exp table: a Sigmoid would force two 1.3us
            ACT_TABLE_LOADs per batch)."""
            col2 = col_t.pop(b)
            p_att = ps_misc.tile([128, 8], F32, tag="psx")
            att = small.tile([NTILE, NT], F32, tag="att")
            for nt in range(NT):
                nc.tensor.matmul(
                    p_att[:NTILE, nt : nt + 1],
                    col2[:, nt * NTILE : (nt + 1) * NTILE],
                    sa_sb[:, 0:1],
                    start=True,
                    stop=True,
                    skip_group_check=True,
                )
            nc.scalar.activation(
                att[:, 0:NT], p_att[:NTILE, 0:NT], AF.Exp, scale=-1.0
            )
            nc.vector.tensor_scalar_add(att[:, 0:NT], att[:, 0:NT], 1.0)
            nc.vector.reciprocal(att[:, 0:NT], att[:, 0:NT])
            # fixed log-domain offset: E stays within exp's fp32 range for
            # any plausible N(0,1)-scaled input (measured margins > 20 both
            # sides), so no per-row max reduction is needed at all
            bias = small.tile([NTILE, NT], F32, tag="bias_t")
            nc.vector.tensor_scalar_mul(bias[:, 0:NT], att[:, 0:NT], -55.0)
            att_t[b] = (att, bias)

        def finish(b):
            """X @ s for batch b: s PSUM->SBUF, 28 tiny matmuls, acc copy."""
            nc.vector.tensor_copy(s_mv[:, 0:NT], pst_t.pop(b)[:, 0:NT])
            xt = xt_t.pop(b)
            p_xs = ps_misc.tile([128, 8], F32, tag="psx")
            for co in range(CCH):
                for mt in range(NT):
                    nc.tensor.matmul(
                        p_xs[:, co : co + 1],
                        xt[:, mt, co * 128 : (co + 1) * 128],
                        s_mv[:, mt : mt + 1],
                        start=(mt == 0),
                        stop=(mt == NT - 1),
                        skip_group_check=True,
                    )
            xs_v = xs_acc[:].rearrange("p (co bb) -> p co bb", bb=BPC)
            nc.vector.tensor_copy(xs_v[:, :, b], p_xs[:, 0:CCH])
            return p_xs

        pst_t = {}
        k_t = {}

        def k_proj(b):
            """k = Wk x (PE, own PSUM pool) + PSUM->SBUF bf16 copies (ACT)."""
            xb = xb_t[b]
            k_sb = kpool.tile([128, CCH, N], BF16, tag="k_sb")
            for co in range(CCH):
                for lo, hi in ((0, NH0), (NH0, N)):
                    pk = kps.tile([128, NH0], F32, tag="pk")
                    for ci in range(CCH):
                        nc.tensor.matmul(
                            pk[:, 0 : hi - lo],
                            wk_sb[:, ci, co * 128 : (co + 1) * 128],
                            xb[:, ci, lo:hi],
                            start=(ci == 0),
                            stop=(ci == CCH - 1),
                        )
                    nc.scalar.copy(k_sb[:, co, lo:hi], pk[:, 0 : hi - lo])
            k_t[b] = k_sb

        # ---- prologue: x loads first, then consts in order of first use ----
        xv0 = xd[0].rearrange("(ci c) n -> c ci n", c=128)
        xb0 = xpool.tile([128, CCH, N], BF16, tag="xb", name="xb0")
        for h in range(2):
            nc.sync.dma_start(
                xb0[:, 2 * h : 2 * h + 2, :], xv0[:, 2 * h : 2 * h + 2, :]
            )
        xb_t[0] = xb0
        nc.sync.dma_start(wk_sb, wkd[:].rearrange("(ci c) o -> c ci o", c=128))
        xt0 = xtp.tile([NTILE, NT, C], BF16, tag="xt", name="xt0")
        xv2 = xtd[0].rearrange("nt p c -> p nt c")
        nc.sync.dma_start(xt0[:, 0:4, :], xv2[:, 0:4, :])
        nc.sync.dma_start(xt0[:, 4:NT, :], xv2[:, 4:NT, :])
        xt_t[0] = xt0
        for fd in fds:
            nc.sync.dma_start(fd, zsb)
        nc.sync.dma_start(id_sb, idd[:])
        nc.sync.dma_start(sa_sb, sad[:])
        load_x(1)
        load_xf(0)
        nc.sync.dma_start(gb_sb, gbd[:])
        nc.sync.dma_start(wv_sb, wvd[:].rearrange("(ci c) o -> c ci o", c=128))
        for j in range(4):
            front_max_tile(0, j)
        front_sum(0)
        k_proj(0)
        front_maxplane(0)
        front_col(0)

        for b in range(BPC):
            xb = xb_t.pop(b)
            if b + 2 < BPC:
                load_x(b + 2)
            if b + 1 < BPC:
                load_xf(b + 1)
            if b > 0:
                finish(b - 1)

            # conv for THIS batch: col(b) was gathered an iteration ago
            front_conv(b)
            k_sb = k_t.pop(b)

            att, bias_t = att_t.pop(b)
            xf = xf_t.pop(b)
            exp_sb = epool.tile([NTILE, NT, N], BF16, tag="exp_sb")
            r_bf = small.tile([NTILE, NT], BF16, tag="r_bf")
            zsum = small.tile([NTILE, NT], F32, tag="zsum")
            p_stile = ps_misc.tile([NTILE, 8], F32, tag="p_stile")
            pst_t[b] = p_stile

            def s_mms(nt):
                # s[m] accumulation, position-major: stationary = exp tile,
                # moving = per-row 1/Z. 7 one-column matmuls.
                for mt in range(NT):
                    nc.tensor.matmul(
                        p_stile[:, mt : mt + 1],
                        exp_sb[:, nt, mt * NTILE : (mt + 1) * NTILE],
                        r_bf[:, nt : nt + 1],
                        start=(nt == 0),
                        stop=(nt == NT - 1),
                        skip_group_check=True,
                    )

            # ---- energy + fused softmax ----
            for nt in range(NT):
                pe = ps_big.tile([128, 1024], F32, tag="pE")
                nsl = slice(nt * NTILE, (nt + 1) * NTILE)
                for lo, hi in ((0, NH0), (NH0, N)):
                    for ci in range(CCH):
                        nc.tensor.matmul(
                            pe[:NTILE, lo:hi],
                            xb[:, ci, nsl],
                            k_sb[:, ci, lo:hi],
                            start=(ci == 0),
                            stop=(ci == CCH - 1),
                        )
                if nt > 1:
                    s_mms(nt - 2)
                if nt == 3 and b + 1 < BPC:
                    front_sum(b + 1)

                nc.scalar.activation(
                    exp_sb[:, nt, :],
                    pe[:NTILE, 0:N],
                    AF.Exp,
                    bias=bias_t[:, nt : nt + 1],
                    scale=att[:, nt : nt + 1],
                    accum_out=zsum[:, nt : nt + 1],
                )
                nc.vector.reciprocal(zsum[:, nt : nt + 1], zsum[:, nt : nt + 1])
                nc.vector.tensor_copy(r_bf[:, nt : nt + 1], zsum[:, nt : nt + 1])
                # next batch's channel-max rides the DVE slack, 1 tile per nt
                if b + 1 < BPC:
                    front_max_tile(b + 1, nt)

            # next batch's K projection fills the PE while exp(nt5)/exp(nt6)
            # and their reciprocals land on ACT/DVE
            if b + 1 < BPC:
                k_proj(b + 1)
            # exact fp32 xmean: only needed by the final tail, so it sits at
            # the very end of the iteration's DVE queue where a late xf DMA
            # cannot stall anything downstream
            for ci in range(CCH):
                acc = xm_acc[:, ci * BPC + b : ci * BPC + b + 1]
                if ci < 2:
                    nc.scalar.activation(
                        xm_dump, xf[:, ci, :], AF.Copy,
                        bias=0.0, scale=1.0 / N, accum_out=acc,
                    )
                else:
                    nc.vector.tensor_scalar(
                        out=xm_dump, in0=xf[:, ci, :],
                        scalar1=1.0 / N, scalar2=0.0,
                        op0=AL.mult, op1=AL.add, accum_out=acc,
                    )
            if b + 1 == BPC:
                # no next-batch K to fill the PE: bridge the exp(nt5)/exp(nt6)
                # latency with warm dummies so HAM stays open into the tail
                for _ in range(56):
                    nc.tensor.matmul(
                        p_warm[0:1, 0:8], ones_bf, warm,
                        start=True, stop=True, skip_group_check=True,
                    )
            s_mms(NT - 2)
            if b + 1 == BPC:
                for _ in range(40):
                    nc.tensor.matmul(
                        p_warm[0:1, 0:8], ones_bf, warm,
                        start=True, stop=True, skip_group_check=True,
                    )
            s_mms(NT - 1)
            if b + 1 < BPC:
                front_maxplane(b + 1)
                front_col(b + 1)

        finish(BPC - 1)

        # ---- tail: res = Wv^T @ XS ; out = res*(gamma/N) + (gamma*bv + xmean)
        res_all = scratch.tile([128, CCH, BPC], F32, tag="res_all")
        for co in range(CCH):
            pr = ps_big.tile([128, 1024], F32, tag="pE")
            for ci in range(CCH):
                nc.tensor.matmul(
                    pr[:, 0:BPC],
                    wv_sb[:, ci, co * 128 : (co + 1) * 128],
                    xs_acc[:, ci * BPC : (ci + 1) * BPC],
                    start=(ci == 0),
                    stop=(ci == CCH - 1),
                    skip_group_check=True,
                )
            t2 = scratch.tile([128, BPC], F32, tag=f"t2_{co}", name="t2")
            nc.vector.tensor_scalar_add(
                t2, xm_acc[:, co * BPC : (co + 1) * BPC], gb_sb[:, co : co + 1]
            )
            nc.vector.scalar_tensor_tensor(
                out=res_all[:, co, :],
                in0=pr[:, 0:BPC],
                scalar=gb_sb[:, CCH : CCH + 1],
                in1=t2,
                op0=AL.mult,
                op1=AL.add,
            )
        nc.sync.dma_start(
            outd[:].rearrange("(ci c) b -> c ci b", c=128), res_all
        )


_CACHE = {}


def _get_nc():
    if "nc" not in _CACHE:
        _CACHE["nc"] = _build()
    return _CACHE["nc"]


def kernel(x, sa_w, key_w, key_b, value_w, value_b, gamma, _trace=False):
    import ml_dtypes

    BF = ml_dtypes.bfloat16
    x = np.ascontiguousarray(np.asarray(x, dtype=np.float32)).reshape(B, C, N)
    sa_w = np.asarray(sa_w, dtype=np.float32)
    key_w = np.asarray(key_w, dtype=np.float32)
    value_w = np.asarray(value_w, dtype=np.float32)
    value_b = np.asarray(value_b, dtype=np.float32)
    gamma = float(np.asarray(gamma).reshape(-1)[0])

    # host-side parameter reshuffles (layout only / tiny folds)
    sa98 = sa_w.reshape(2, KS * KS).copy()
    sa98[0] *= 1.0 / C                      # channel-mean fold
    sa98 = np.repeat(sa98.reshape(TAPS, 1), 16, axis=1).astype(BF)
    sa98 = np.ascontiguousarray(sa98)
    wkT = np.ascontiguousarray(key_w.T.astype(BF))
    wvT = np.ascontiguousarray(value_w.T.astype(BF))
    gbvg = np.empty((128, CCH + 1), np.float32)
    gbvg[:, :CCH] = (gamma * value_b).reshape(CCH, 128).T
    gbvg[:, CCH] = gamma / N
    gbvg = np.ascontiguousarray(gbvg)
    ident = np.eye(128, dtype=np.float32).astype(BF)

    xbf = x.astype(BF)                                        # [B, C, N]
    xt = np.ascontiguousarray(
        xbf.reshape(B, C, NT, NTILE).transpose(0, 2, 3, 1)
    )                                                         # [B, NT, 112, C]

    nc = _get_nc()
    in_maps = []
    for i in range(NCORES):
        sl = slice(i * BPC, (i + 1) * BPC)
        in_maps.append(
            {
                "x": np.ascontiguousarray(xbf[sl]),
                "xf": np.ascontiguousarray(x[sl]),
                "xt": xt[sl],
                "wkT": wkT,
                "wvT": wvT,
                "sa98": sa98,
                "gbvg": gbvg,
                "ident": ident,
            }
        )
    r = run_bass_kernel_spmd(
        nc, in_maps, core_ids=list(range(NCORES)), trace=_trace
    )
    out = np.empty((B, C), np.float32)
    for i in range(NCORES):
        out[i * BPC : (i + 1) * BPC] = r.results[i]["out"].T
    if _trace:
        kernel.last_results = r
    return out
